# revision 12
# baseline (speedup 1.0000x reference)
"""MiniBatchDiscrimination Trainium2 kernel.

reference:
    proj = x @ W.T                      # [512, 500] -> [512, 100, 5]
    l1[i,j,o] = sum_k |proj[i,o,k] - proj[j,o,k]|
    mbd[i,o]  = sum_j exp(-l1[i,j,o]) - 1
    out = concat([x, mbd], axis=1)      # [512, 1124]

Strategy (8 cores, shard i-rows of the BxB pairwise computation):
  - Host passes x.T (per-core column-rotated so that the core's 64 local
    rows sit in columns 0..63) and W.T with rows permuted k-major, so one
    SPMD program serves all cores with zero device-side core-id logic.
  - Inputs are fp16: PE matmul runs full-rate single pass (~5e-4 precision),
    and the whole input load is 2 MB per core.
  - proj.T [500, 512] per core via PE matmul, kept as fp16 tiles [125, 512]
    x4 for the pairwise stage + small fp32 [125, 64] local-column blocks
    for bias/scalar operands.
  - Pairwise stage per local row i (A-quad [125, 4, 512] fp16):
      sub slice t:  ScalarE Abs(-projTb + bias_col)  (fused abs)
                or  VectorE tensor_scalar(sub)       (2x mode)
      one VectorE bitwise-AND 0x7FFF over the whole quad [125, 2048]
        clears fp16 sign bits -> |d| (idempotent on ACT-produced slices).
      k-reduce: PE matmul, 0/1 selector S_t [125, 100] fp16, contracting
        the partition axis, 4 slices accumulating into PSUM [100, 512].
      exp + j-reduce: one ScalarE Exp(scale=-1) reading PSUM, accum_out
        writes the free-axis sum straight into mbdT[:, i].
  - Host assembles: mbd = gather(mbdT).T - 1; out = [x | mbd].
"""

import sys

import numpy as np

sys.path.insert(0, "/opt/trn_rl_repo")

import concourse.bacc as bacc  # noqa: E402
import concourse.mybir as mybir  # noqa: E402
import concourse.tile as tile  # noqa: E402
from concourse.bass_utils import run_bass_kernel_spmd  # noqa: E402

B, IN, O, K = 512, 1024, 100, 5
OK = O * K  # 500
NCORES = 8
BL = B // NCORES  # 64 local rows per core
NT = 4  # proj.T partition tiles
PT = OK // NT  # 125 partitions per tile
NIN = IN // 128  # 8 contraction chunks

F32 = mybir.dt.float32
F16 = mybir.dt.float16
U16 = mybir.dt.uint16
AF = mybir.ActivationFunctionType
ALU = mybir.AluOpType

# of every 9 i-rows, this many have their whole absdiff quad on ScalarE
# (fused abs, no AND pass); the rest run on VectorE (sub + one quad AND)
ACT_QUADS_PER_9 = 2
GSZ = 4  # i-rows per PSUM group; 2 groups pipeline across the 8 banks


def build():
    nc = bacc.Bacc("TRN2", target_bir_lowering=False)
    xT_d = nc.dram_tensor("xT", [IN, B], F16, kind="ExternalInput")
    wT_d = nc.dram_tensor("wT", [IN, OK], F16, kind="ExternalInput")
    sel = nc.dram_tensor("sel", [NT, PT, O], F16, kind="ExternalInput")
    mbdT_d = nc.dram_tensor("mbdT", [O, BL], F32, kind="ExternalOutput")

    with tile.TileContext(nc) as tc:
        with (
            tc.tile_pool(name="pers", bufs=1) as pers,
            tc.tile_pool(name="io", bufs=NIN) as io,
            tc.tile_pool(name="work", bufs=3) as work,
            tc.tile_pool(name="esc", bufs=3) as esc,
            tc.tile_pool(name="ps", bufs=8, space="PSUM") as ps,
        ):
            # selector matrices (0/1), one per ok-tile
            s_sb = []
            for t in range(NT):
                s_t = pers.tile([PT, O], F16, name=f"s{t}", tag=f"s{t}")
                nc.sync.dma_start(out=s_t[:], in_=sel[t])
                s_sb.append(s_t)

            # persistent proj.T tiles (fp16 full + fp32 local cols) and output
            projTb = [
                pers.tile([PT, B], F16, name=f"projTb{t}", tag=f"projTb{t}")
                for t in range(NT)
            ]
            projL = [
                pers.tile([PT, BL], F32, name=f"projL{t}", tag=f"projL{t}")
                for t in range(NT)
            ]
            mbdT_sb = pers.tile([O, BL], F32, name="mbdT_sb", tag="mbdT_sb")

            # ---- proj phase: proj.T[p, j] = sum_in wT[in, p] * xT[in, j] ----
            pps = [ps.tile([PT, B], F32, name=f"pps{t}", tag="ps") for t in range(NT)]
            xcs, wcs = [], []
            for c in range(NIN):
                x_c = io.tile([128, B], F16, name=f"x{c}", tag="xc")
                nc.sync.dma_start(out=x_c[:], in_=xT_d[128 * c : 128 * (c + 1), :])
                xcs.append(x_c)
                w_c = io.tile([128, OK], F16, name=f"w{c}", tag="wc")
                nc.sync.dma_start(out=w_c[:], in_=wT_d[128 * c : 128 * (c + 1), :])
                wcs.append(w_c)
            for t in range(NT):
                for c in range(NIN):
                    nc.tensor.matmul(
                        pps[t][:],
                        lhsT=wcs[c][:, PT * t : PT * (t + 1)],
                        rhs=xcs[c][:],
                        start=(c == 0),
                        stop=(c == NIN - 1),
                    )
                nc.vector.tensor_copy(projTb[t][:], pps[t][:])
                nc.scalar.copy(projL[t][:], pps[t][:, :BL])

            # ---- pairwise phase ----
            for g0 in range(0, BL, GSZ):
                gis = range(g0, min(g0 + GSZ, BL))
                psums = {
                    i: ps.tile([O, B], F32, name=f"ps{i}", tag="ps") for i in gis
                }
                for i in gis:
                    aq = work.tile([PT, NT, B], F16, name=f"a{i}", tag="A")
                    on_act = (i % 9) < ACT_QUADS_PER_9
                    for t in range(NT):
                        col = projL[t][:, i : i + 1]
                        if on_act:
                            nc.scalar.activation(
                                out=aq[:, t, :],
                                in_=projTb[t][:],
                                func=AF.Abs,
                                bias=col,
                                scale=-1.0,
                            )
                        else:
                            nc.vector.tensor_scalar(
                                aq[:, t, :],
                                projTb[t][:],
                                col,
                                None,
                                op0=ALU.subtract,
                            )
                    if not on_act:
                        nc.vector.tensor_scalar(
                            aq[:].bitcast(U16),
                            aq[:].bitcast(U16),
                            0x7FFF,
                            None,
                            op0=ALU.bitwise_and,
                        )
                    for t in range(NT):
                        nc.tensor.matmul(
                            psums[i][:],
                            lhsT=s_sb[t][:],
                            rhs=aq[:, t, :],
                            start=(t == 0),
                            stop=(t == NT - 1),
                        )
                for i in gis:
                    e = esc.tile([O, B], F16, name=f"e{i}", tag="E")
                    nc.scalar.activation(
                        out=e[:],
                        in_=psums[i][:],
                        func=AF.Exp,
                        scale=-1.0,
                        accum_out=mbdT_sb[:, i : i + 1],
                    )

            nc.sync.dma_start(out=mbdT_d[:, :], in_=mbdT_sb[:])
    nc.compile()
    return nc


_CACHE = {}


def _build_cached():
    if "nc" not in _CACHE:
        _CACHE["nc"] = build()
    return _CACHE["nc"]


def _selector() -> np.ndarray:
    sel = np.zeros((NT, PT, O), np.float32)
    for t in range(NT):
        for p in range(PT):
            sel[t, p, (t * PT + p) % O] = 1.0
    return sel.astype(np.float16)


def make_in_maps(x: np.ndarray, W: np.ndarray):
    xT = np.ascontiguousarray(x.T.astype(np.float16))  # [IN, B]
    # k-major proj.T rows: row p corresponds to (o = p % O, k = p // O),
    # i.e. W row o*K + k
    perm = np.array([(p % O) * K + p // O for p in range(OK)], np.int64)
    wTk = np.ascontiguousarray(W.T.astype(np.float16)[:, perm])  # [IN, OK]
    sel = _selector()
    in_maps = []
    for r in range(NCORES):
        in_maps.append(
            {
                "xT": np.ascontiguousarray(np.roll(xT, -BL * r, axis=1)),
                "wT": wTk,
                "sel": sel,
            }
        )
    return in_maps


def run(x, W, trace=False, **kw):
    nc = _build_cached()
    in_maps = make_in_maps(x, W)
    return run_bass_kernel_spmd(
        nc, in_maps, core_ids=list(range(NCORES)), trace=trace, **kw
    )


def kernel(x: np.ndarray, W: np.ndarray) -> np.ndarray:
    x = np.asarray(x, np.float32)
    W = np.asarray(W, np.float32)
    res = run(x, W, trace=False)
    mbd = np.empty((B, O), np.float32)
    for r in range(NCORES):
        mbd[BL * r : BL * (r + 1), :] = res.results[r]["mbdT"].T
    mbd -= 1.0
    return np.concatenate([x, mbd], axis=1)


# revision 13
# speedup vs baseline: 1.0105x; 1.0105x over previous
"""MiniBatchDiscrimination Trainium2 kernel.

reference:
    proj = x @ W.T                      # [512, 500] -> [512, 100, 5]
    l1[i,j,o] = sum_k |proj[i,o,k] - proj[j,o,k]|
    mbd[i,o]  = sum_j exp(-l1[i,j,o]) - 1
    out = concat([x, mbd], axis=1)      # [512, 1124]

Strategy (8 cores, shard i-rows of the BxB pairwise computation):
  - Host passes x.T (per-core column-rotated so that the core's 64 local
    rows sit in columns 0..63) and W.T with rows permuted k-major, so one
    SPMD program serves all cores with zero device-side core-id logic.
  - Inputs are fp16: PE matmul runs full-rate single pass (~5e-4 precision),
    and the whole input load is 2 MB per core.
  - proj.T [500, 512] per core via PE matmul, kept as fp16 tiles [125, 512]
    x4 for the pairwise stage + small fp32 [125, 64] local-column blocks
    for bias/scalar operands.
  - Pairwise stage per local row i (A-quad [125, 4, 512] fp16):
      sub slice t:  ScalarE Abs(-projTb + bias_col)  (fused abs)
                or  VectorE tensor_scalar(sub)       (2x mode)
      one VectorE bitwise-AND 0x7FFF over the whole quad [125, 2048]
        clears fp16 sign bits -> |d| (idempotent on ACT-produced slices).
      k-reduce: PE matmul, 0/1 selector S_t [125, 100] fp16, contracting
        the partition axis, 4 slices accumulating into PSUM [100, 512].
      exp + j-reduce: one ScalarE Exp(scale=-1) reading PSUM, accum_out
        writes the free-axis sum straight into mbdT[:, i].
  - Host assembles: mbd = gather(mbdT).T - 1; out = [x | mbd].
"""

import sys

import numpy as np

sys.path.insert(0, "/opt/trn_rl_repo")

import concourse.bacc as bacc  # noqa: E402
import concourse.mybir as mybir  # noqa: E402
import concourse.tile as tile  # noqa: E402
from concourse.bass_utils import run_bass_kernel_spmd  # noqa: E402

B, IN, O, K = 512, 1024, 100, 5
OK = O * K  # 500
NCORES = 8
BL = B // NCORES  # 64 local rows per core
NT = 4  # proj.T partition tiles
PT = OK // NT  # 125 partitions per tile
NIN = IN // 128  # 8 contraction chunks

F32 = mybir.dt.float32
F16 = mybir.dt.float16
U16 = mybir.dt.uint16
AF = mybir.ActivationFunctionType
ALU = mybir.AluOpType

# of every 9 i-rows, this many have their whole absdiff quad on ScalarE
# (fused abs, no AND pass); the rest run on VectorE (sub + one quad AND)
ACT_QUADS_PER_9 = 2
GSZ = 4  # i-rows per PSUM group; 2 groups pipeline across the 8 banks


def build():
    nc = bacc.Bacc("TRN2", target_bir_lowering=False)
    xT_d = nc.dram_tensor("xT", [IN, B], F16, kind="ExternalInput")
    wT_d = nc.dram_tensor("wT", [IN, OK], F16, kind="ExternalInput")
    sel = nc.dram_tensor("sel", [NT, PT, O], F16, kind="ExternalInput")
    mbdT_d = nc.dram_tensor("mbdT", [O, BL], F32, kind="ExternalOutput")

    with tile.TileContext(nc) as tc:
        with (
            tc.tile_pool(name="pers", bufs=1) as pers,
            tc.tile_pool(name="io", bufs=NIN) as io,
            tc.tile_pool(name="work", bufs=8) as work,
            tc.tile_pool(name="esc", bufs=4) as esc,
            tc.tile_pool(name="ps", bufs=8, space="PSUM") as ps,
        ):
            # selector matrices (0/1), one per ok-tile
            s_sb = []
            for t in range(NT):
                s_t = pers.tile([PT, O], F16, name=f"s{t}", tag=f"s{t}")
                nc.sync.dma_start(out=s_t[:], in_=sel[t])
                s_sb.append(s_t)

            # persistent proj.T tiles (fp16 full + fp32 local cols) and output
            projTb = [
                pers.tile([PT, B], F16, name=f"projTb{t}", tag=f"projTb{t}")
                for t in range(NT)
            ]
            projL = [
                pers.tile([PT, BL], F32, name=f"projL{t}", tag=f"projL{t}")
                for t in range(NT)
            ]
            mbdT_sb = pers.tile([O, BL], F32, name="mbdT_sb", tag="mbdT_sb")

            # ---- proj phase: proj.T[p, j] = sum_in wT[in, p] * xT[in, j] ----
            pps = [ps.tile([PT, B], F32, name=f"pps{t}", tag="ps") for t in range(NT)]
            xcs = []
            for c in range(NIN):
                x_c = io.tile([128, B], F16, name=f"x{c}", tag="xc")
                nc.sync.dma_start(out=x_c[:], in_=xT_d[128 * c : 128 * (c + 1), :])
                xcs.append(x_c)
            # W loads column-sliced per ok-tile so chain t only waits on its
            # own 8 slices (the first pairwise work starts ~1.25 MB in, not 2 MB)
            wts = []
            for t in range(NT):
                per_c = []
                for c in range(NIN):
                    w_ct = io.tile([128, PT], F16, name=f"w{c}_{t}", tag=f"wc{t}")
                    nc.sync.dma_start(
                        out=w_ct[:],
                        in_=wT_d[128 * c : 128 * (c + 1), PT * t : PT * (t + 1)],
                    )
                    per_c.append(w_ct)
                wts.append(per_c)
            for t in range(NT):
                for c in range(NIN):
                    nc.tensor.matmul(
                        pps[t][:],
                        lhsT=wts[t][c][:],
                        rhs=xcs[c][:],
                        start=(c == 0),
                        stop=(c == NIN - 1),
                    )
                nc.vector.tensor_copy(projTb[t][:], pps[t][:])
                nc.scalar.copy(projL[t][:], pps[t][:, :BL])

            # ---- pairwise phase ----
            for g0 in range(0, BL, GSZ):
                gis = range(g0, min(g0 + GSZ, BL))
                psums = {
                    i: ps.tile([O, B], F32, name=f"ps{i}", tag="ps") for i in gis
                }
                for i in gis:
                    aq = work.tile([PT, NT, B], F16, name=f"a{i}", tag="A")
                    on_act = (i % 9) < ACT_QUADS_PER_9
                    for t in range(NT):
                        col = projL[t][:, i : i + 1]
                        if on_act:
                            nc.scalar.activation(
                                out=aq[:, t, :],
                                in_=projTb[t][:],
                                func=AF.Abs,
                                bias=col,
                                scale=-1.0,
                            )
                        else:
                            nc.vector.tensor_scalar(
                                aq[:, t, :],
                                projTb[t][:],
                                col,
                                None,
                                op0=ALU.subtract,
                            )
                    if not on_act:
                        nc.vector.tensor_scalar(
                            aq[:].bitcast(U16),
                            aq[:].bitcast(U16),
                            0x7FFF,
                            None,
                            op0=ALU.bitwise_and,
                        )
                    for t in range(NT):
                        nc.tensor.matmul(
                            psums[i][:],
                            lhsT=s_sb[t][:],
                            rhs=aq[:, t, :],
                            start=(t == 0),
                            stop=(t == NT - 1),
                        )
                for i in gis:
                    e = esc.tile([O, B], F16, name=f"e{i}", tag="E")
                    nc.scalar.activation(
                        out=e[:],
                        in_=psums[i][:],
                        func=AF.Exp,
                        scale=-1.0,
                        accum_out=mbdT_sb[:, i : i + 1],
                    )

            nc.sync.dma_start(out=mbdT_d[:, :], in_=mbdT_sb[:])
    nc.compile()
    return nc


_CACHE = {}


def _build_cached():
    if "nc" not in _CACHE:
        _CACHE["nc"] = build()
    return _CACHE["nc"]


def _selector() -> np.ndarray:
    sel = np.zeros((NT, PT, O), np.float32)
    for t in range(NT):
        for p in range(PT):
            sel[t, p, (t * PT + p) % O] = 1.0
    return sel.astype(np.float16)


def make_in_maps(x: np.ndarray, W: np.ndarray):
    xT = np.ascontiguousarray(x.T.astype(np.float16))  # [IN, B]
    # k-major proj.T rows: row p corresponds to (o = p % O, k = p // O),
    # i.e. W row o*K + k
    perm = np.array([(p % O) * K + p // O for p in range(OK)], np.int64)
    wTk = np.ascontiguousarray(W.T.astype(np.float16)[:, perm])  # [IN, OK]
    sel = _selector()
    in_maps = []
    for r in range(NCORES):
        in_maps.append(
            {
                "xT": np.ascontiguousarray(np.roll(xT, -BL * r, axis=1)),
                "wT": wTk,
                "sel": sel,
            }
        )
    return in_maps


def run(x, W, trace=False, **kw):
    nc = _build_cached()
    in_maps = make_in_maps(x, W)
    return run_bass_kernel_spmd(
        nc, in_maps, core_ids=list(range(NCORES)), trace=trace, **kw
    )


def kernel(x: np.ndarray, W: np.ndarray) -> np.ndarray:
    x = np.asarray(x, np.float32)
    W = np.asarray(W, np.float32)
    res = run(x, W, trace=False)
    mbd = np.empty((B, O), np.float32)
    for r in range(NCORES):
        mbd[BL * r : BL * (r + 1), :] = res.results[r]["mbdT"].T
    mbd -= 1.0
    return np.concatenate([x, mbd], axis=1)


# revision 14
# speedup vs baseline: 1.1019x; 1.0905x over previous
"""MiniBatchDiscrimination Trainium2 kernel.

reference:
    proj = x @ W.T                      # [512, 500] -> [512, 100, 5]
    l1[i,j,o] = sum_k |proj[i,o,k] - proj[j,o,k]|
    mbd[i,o]  = sum_j exp(-l1[i,j,o]) - 1
    out = concat([x, mbd], axis=1)      # [512, 1124]

Strategy (8 cores, shard i-rows of the BxB pairwise computation):
  - Host passes x.T (per-core column-rotated so that the core's 64 local
    rows sit in columns 0..63) and W.T with rows permuted k-major, so one
    SPMD program serves all cores with zero device-side core-id logic.
  - Inputs are fp16: PE matmul runs full-rate single pass (~5e-4 precision),
    and the whole input load is 2 MB per core.
  - proj.T [500, 512] per core via PE matmul, kept as fp16 tiles [125, 512]
    x4 for the pairwise stage + small fp32 [125, 64] local-column blocks
    for bias/scalar operands.
  - Pairwise stage per local row i (A-quad [125, 4, 512] fp16):
      sub slice t:  ScalarE Abs(-projTb + bias_col)  (fused abs)
                or  VectorE tensor_scalar(sub)       (2x mode)
      one VectorE bitwise-AND 0x7FFF over the whole quad [125, 2048]
        clears fp16 sign bits -> |d| (idempotent on ACT-produced slices).
      k-reduce: PE matmul, 0/1 selector S_t [125, 100] fp16, contracting
        the partition axis, 4 slices accumulating into PSUM [100, 512].
      exp + j-reduce: one ScalarE Exp(scale=-1) reading PSUM, accum_out
        writes the free-axis sum straight into mbdT[:, i].
  - Host assembles: mbd = gather(mbdT).T - 1; out = [x | mbd].
"""

import sys

import numpy as np

sys.path.insert(0, "/opt/trn_rl_repo")

import concourse.bacc as bacc  # noqa: E402
import concourse.mybir as mybir  # noqa: E402
import concourse.tile as tile  # noqa: E402
from concourse.bass_utils import run_bass_kernel_spmd  # noqa: E402

B, IN, O, K = 512, 1024, 100, 5
OK = O * K  # 500
NCORES = 8
BL = B // NCORES  # 64 local rows per core
NT = 4  # proj.T partition tiles
PT = OK // NT  # 125 partitions per tile
NIN = IN // 128  # 8 contraction chunks

F32 = mybir.dt.float32
F16 = mybir.dt.float16
U16 = mybir.dt.uint16
AF = mybir.ActivationFunctionType
ALU = mybir.AluOpType

# of every 9 i-rows, this many have their whole absdiff quad on ScalarE
# (fused abs, no AND pass); the rest run on VectorE (sub + one quad AND)
ACT_QUADS_PER_9 = 2  # (renamed semantics below use %16)
GSZ = 4  # i-rows per PSUM group; 2 groups pipeline across the 8 banks


def build():
    nc = bacc.Bacc("TRN2", target_bir_lowering=False)
    xT_d = nc.dram_tensor("xT", [IN, B], F16, kind="ExternalInput")
    wT_d = nc.dram_tensor("wT", [IN, OK], F16, kind="ExternalInput")
    sel = nc.dram_tensor("sel", [NT, PT, O], F16, kind="ExternalInput")
    mbdT_d = nc.dram_tensor("mbdT", [O, BL], F32, kind="ExternalOutput")

    with tile.TileContext(nc) as tc:
        with (
            tc.tile_pool(name="pers", bufs=1) as pers,
            tc.tile_pool(name="io", bufs=1) as io,
            tc.tile_pool(name="work", bufs=8) as work,
            tc.tile_pool(name="ps", bufs=8, space="PSUM") as ps,
        ):
            # selector matrices (0/1), one per ok-tile
            s_sb = []
            for t in range(NT):
                s_t = pers.tile([PT, O], F16, name=f"s{t}", tag=f"s{t}")
                nc.sync.dma_start(out=s_t[:], in_=sel[t])
                s_sb.append(s_t)

            # persistent proj.T tiles (fp16 full + fp32 local cols) and output
            projTb = [
                pers.tile([PT, B], F16, name=f"projTb{t}", tag=f"projTb{t}")
                for t in range(NT)
            ]
            projL = [
                pers.tile([PT, BL], F32, name=f"projL{t}", tag=f"projL{t}")
                for t in range(NT)
            ]
            mbdT_sb = pers.tile([O, BL], F32, name="mbdT_sb", tag="mbdT_sb")

            # ---- proj phase: proj.T[p, j] = sum_in wT[in, p] * xT[in, j] ----
            pps = [ps.tile([PT, B], F32, name=f"pps{t}", tag="ps") for t in range(NT)]
            # whole-tensor loads as [128, chunk, row] with 1KB-row descriptors;
            # two dma_starts per tensor to spread across HWDGE queues
            xcat = io.tile([128, NIN, B], F16, name="xcat", tag="xcat")
            wcat = io.tile([128, NIN, OK], F16, name="wcat", tag="wcat")
            xT_v = xT_d[:, :].rearrange("(c p) j -> p c j", p=128)
            wT_v = wT_d[:, :].rearrange("(c p) j -> p c j", p=128)
            h = NIN // 2
            nc.sync.dma_start(out=xcat[:, :h, :], in_=xT_v[:, :h, :])
            nc.sync.dma_start(out=xcat[:, h:, :], in_=xT_v[:, h:, :])
            nc.sync.dma_start(out=wcat[:, :h, :], in_=wT_v[:, :h, :])
            nc.sync.dma_start(out=wcat[:, h:, :], in_=wT_v[:, h:, :])
            for t in range(NT):
                for c in range(NIN):
                    nc.tensor.matmul(
                        pps[t][:],
                        lhsT=wcat[:, c, PT * t : PT * (t + 1)],
                        rhs=xcat[:, c, :],
                        start=(c == 0),
                        stop=(c == NIN - 1),
                    )
                nc.vector.tensor_copy(projTb[t][:], pps[t][:])
                nc.scalar.copy(projL[t][:], pps[t][:, :BL])

            # ---- pairwise phase ----
            for g0 in range(0, BL, GSZ):
                gis = range(g0, min(g0 + GSZ, BL))
                psums = {
                    i: ps.tile([O, B], F32, name=f"ps{i}", tag="ps") for i in gis
                }
                for i in gis:
                    aq = work.tile([PT, NT, B], F16, name=f"a{i}", tag="A")
                    on_act = (i % 16) < 3
                    for t in range(NT):
                        col = projL[t][:, i : i + 1]
                        if on_act:
                            nc.scalar.activation(
                                out=aq[:, t, :],
                                in_=projTb[t][:],
                                func=AF.Abs,
                                bias=col,
                                scale=-1.0,
                            )
                        else:
                            nc.vector.tensor_scalar(
                                aq[:, t, :],
                                projTb[t][:],
                                col,
                                None,
                                op0=ALU.subtract,
                            )
                    if not on_act:
                        nc.vector.tensor_scalar(
                            aq[:].bitcast(U16),
                            aq[:].bitcast(U16),
                            0x7FFF,
                            None,
                            op0=ALU.bitwise_and,
                        )
                    for t in range(NT):
                        nc.tensor.matmul(
                            psums[i][:],
                            lhsT=s_sb[t][:],
                            rhs=aq[:, t, :],
                            start=(t == 0),
                            stop=(t == NT - 1),
                        )
                for i in gis:
                    nc.scalar.activation(
                        out=psums[i][:],
                        in_=psums[i][:],
                        func=AF.Exp,
                        scale=-1.0,
                        accum_out=mbdT_sb[:, i : i + 1],
                    )

            nc.sync.dma_start(out=mbdT_d[:, :], in_=mbdT_sb[:])
    nc.compile()
    return nc


_CACHE = {}


def _build_cached():
    if "nc" not in _CACHE:
        _CACHE["nc"] = build()
    return _CACHE["nc"]


def _selector() -> np.ndarray:
    sel = np.zeros((NT, PT, O), np.float32)
    for t in range(NT):
        for p in range(PT):
            sel[t, p, (t * PT + p) % O] = 1.0
    return sel.astype(np.float16)


def make_in_maps(x: np.ndarray, W: np.ndarray):
    xT = np.ascontiguousarray(x.T.astype(np.float16))  # [IN, B]
    # k-major proj.T rows: row p corresponds to (o = p % O, k = p // O),
    # i.e. W row o*K + k
    perm = np.array([(p % O) * K + p // O for p in range(OK)], np.int64)
    wTk = np.ascontiguousarray(W.T.astype(np.float16)[:, perm])  # [IN, OK]
    sel = _selector()
    in_maps = []
    for r in range(NCORES):
        in_maps.append(
            {
                "xT": np.ascontiguousarray(np.roll(xT, -BL * r, axis=1)),
                "wT": wTk,
                "sel": sel,
            }
        )
    return in_maps


def run(x, W, trace=False, **kw):
    nc = _build_cached()
    in_maps = make_in_maps(x, W)
    return run_bass_kernel_spmd(
        nc, in_maps, core_ids=list(range(NCORES)), trace=trace, **kw
    )


def kernel(x: np.ndarray, W: np.ndarray) -> np.ndarray:
    x = np.asarray(x, np.float32)
    W = np.asarray(W, np.float32)
    res = run(x, W, trace=False)
    mbd = np.empty((B, O), np.float32)
    for r in range(NCORES):
        mbd[BL * r : BL * (r + 1), :] = res.results[r]["mbdT"].T
    mbd -= 1.0
    return np.concatenate([x, mbd], axis=1)


# revision 16
# speedup vs baseline: 1.1134x; 1.0104x over previous
"""MiniBatchDiscrimination Trainium2 kernel.

reference:
    proj = x @ W.T                      # [512, 500] -> [512, 100, 5]
    l1[i,j,o] = sum_k |proj[i,o,k] - proj[j,o,k]|
    mbd[i,o]  = sum_j exp(-l1[i,j,o]) - 1
    out = concat([x, mbd], axis=1)      # [512, 1124]

Strategy (8 cores, shard i-rows of the BxB pairwise computation):
  - Host passes x.T (per-core column-rotated so that the core's 64 local
    rows sit in columns 0..63) and W.T with rows permuted k-major, so one
    SPMD program serves all cores with zero device-side core-id logic.
  - Inputs are fp16: PE matmul runs full-rate single pass (~5e-4 precision),
    and the whole input load is 2 MB per core.
  - proj.T [500, 512] per core via PE matmul, kept as fp16 tiles [125, 512]
    x4 for the pairwise stage + small fp32 [125, 64] local-column blocks
    for bias/scalar operands.
  - Pairwise stage per local row i (A-quad [125, 4, 512] fp16):
      sub slice t:  ScalarE Abs(-projTb + bias_col)  (fused abs)
                or  VectorE tensor_scalar(sub)       (2x mode)
      one VectorE bitwise-AND 0x7FFF over the whole quad [125, 2048]
        clears fp16 sign bits -> |d| (idempotent on ACT-produced slices).
      k-reduce: PE matmul, 0/1 selector S_t [125, 100] fp16, contracting
        the partition axis, 4 slices accumulating into PSUM [100, 512].
      exp + j-reduce: one ScalarE Exp(scale=-1) reading PSUM, accum_out
        writes the free-axis sum straight into mbdT[:, i].
  - Host assembles: mbd = gather(mbdT).T - 1; out = [x | mbd].
"""

import sys

import numpy as np

sys.path.insert(0, "/opt/trn_rl_repo")

import concourse.bacc as bacc  # noqa: E402
import concourse.mybir as mybir  # noqa: E402
import concourse.tile as tile  # noqa: E402
from concourse.bass_utils import run_bass_kernel_spmd  # noqa: E402

B, IN, O, K = 512, 1024, 100, 5
OK = O * K  # 500
NCORES = 8
BL = B // NCORES  # 64 local rows per core
NT = 4  # proj.T partition tiles
PT = OK // NT  # 125 partitions per tile
NIN = IN // 128  # 8 contraction chunks

F32 = mybir.dt.float32
F16 = mybir.dt.float16
U16 = mybir.dt.uint16
AF = mybir.ActivationFunctionType
ALU = mybir.AluOpType

# of every 9 i-rows, this many have their whole absdiff quad on ScalarE
# (fused abs, no AND pass); the rest run on VectorE (sub + one quad AND)
ACT_QUADS_PER_9 = 2  # (renamed semantics below use %16)
FUSED_ABS = False  # walrus rejects arith op0 + bitwise op1
GSZ = 4  # i-rows per PSUM group; 2 groups pipeline across the 8 banks


def build():
    nc = bacc.Bacc("TRN2", target_bir_lowering=False)
    xT_d = nc.dram_tensor("xT", [128, NIN * B], F16, kind="ExternalInput")
    wT_d = nc.dram_tensor("wT", [128, NIN * OK], F16, kind="ExternalInput")
    sel = nc.dram_tensor("sel", [NT, PT, O], F16, kind="ExternalInput")
    mbdT_d = nc.dram_tensor("mbdT", [O, BL], F32, kind="ExternalOutput")

    with tile.TileContext(nc) as tc:
        with (
            tc.tile_pool(name="pers", bufs=1) as pers,
            tc.tile_pool(name="io", bufs=1) as io,
            tc.tile_pool(name="work", bufs=8) as work,
            tc.tile_pool(name="ps", bufs=8, space="PSUM") as ps,
        ):
            # selector matrices (0/1), one per ok-tile
            s_sb = []
            for t in range(NT):
                s_t = pers.tile([PT, O], F16, name=f"s{t}", tag=f"s{t}")
                nc.sync.dma_start(out=s_t[:], in_=sel[t])
                s_sb.append(s_t)

            # persistent proj.T tiles (fp16 full + fp32 local cols) and output
            projTb = [
                pers.tile([PT, B], F16, name=f"projTb{t}", tag=f"projTb{t}")
                for t in range(NT)
            ]
            projL = [
                pers.tile([PT, BL], F32, name=f"projL{t}", tag=f"projL{t}")
                for t in range(NT)
            ]
            mbdT_sb = pers.tile([O, BL], F32, name="mbdT_sb", tag="mbdT_sb")

            # ---- proj phase: proj.T[p, j] = sum_in wT[in, p] * xT[in, j] ----
            pps = [ps.tile([PT, B], F32, name=f"pps{t}", tag="ps") for t in range(NT)]
            # whole-tensor loads as [128, chunk, row] with 1KB-row descriptors;
            # two dma_starts per tensor to spread across HWDGE queues
            xcat = io.tile([128, NIN, B], F16, name="xcat", tag="xcat")
            wcat = io.tile([128, NIN, OK], F16, name="wcat", tag="wcat")
            h = NIN // 2
            nc.sync.dma_start(out=xcat[:, :h, :], in_=xT_d[:, : h * B])
            nc.sync.dma_start(out=xcat[:, h:, :], in_=xT_d[:, h * B :])
            nc.sync.dma_start(out=wcat[:, :h, :], in_=wT_d[:, : h * OK])
            nc.sync.dma_start(out=wcat[:, h:, :], in_=wT_d[:, h * OK :])
            for t in range(NT):
                for c in range(NIN):
                    nc.tensor.matmul(
                        pps[t][:],
                        lhsT=wcat[:, c, PT * t : PT * (t + 1)],
                        rhs=xcat[:, c, :],
                        start=(c == 0),
                        stop=(c == NIN - 1),
                    )
                nc.vector.tensor_copy(projTb[t][:], pps[t][:])
                nc.scalar.copy(projL[t][:], pps[t][:, :BL])

            # ---- pairwise phase ----
            for g0 in range(0, BL, GSZ):
                gis = range(g0, min(g0 + GSZ, BL))
                psums = {
                    i: ps.tile([O, B], F32, name=f"ps{i}", tag="ps") for i in gis
                }
                for i in gis:
                    aq = work.tile([PT, NT, B], F16, name=f"a{i}", tag="A")
                    on_act = (i % 16) < 3
                    for t in range(NT):
                        col = projL[t][:, i : i + 1]
                        if on_act:
                            nc.scalar.activation(
                                out=aq[:, t, :],
                                in_=projTb[t][:],
                                func=AF.Abs,
                                bias=col,
                                scale=-1.0,
                            )
                        else:
                            if FUSED_ABS:
                                nc.vector.tensor_scalar(
                                    aq[:, t, :],
                                    projTb[t][:],
                                    col,
                                    0x7FFFFFFF,
                                    op0=ALU.subtract,
                                    op1=ALU.bitwise_and,
                                )
                            else:
                                nc.vector.tensor_scalar(
                                    aq[:, t, :],
                                    projTb[t][:],
                                    col,
                                    None,
                                    op0=ALU.subtract,
                                )
                    if not on_act and not FUSED_ABS:
                        nc.vector.tensor_scalar(
                            aq[:].bitcast(U16),
                            aq[:].bitcast(U16),
                            0x7FFF,
                            None,
                            op0=ALU.bitwise_and,
                        )
                    for t in range(NT):
                        nc.tensor.matmul(
                            psums[i][:],
                            lhsT=s_sb[t][:],
                            rhs=aq[:, t, :],
                            start=(t == 0),
                            stop=(t == NT - 1),
                        )
                for i in gis:
                    nc.scalar.activation(
                        out=psums[i][:],
                        in_=psums[i][:],
                        func=AF.Exp,
                        scale=-1.0,
                        accum_out=mbdT_sb[:, i : i + 1],
                    )

            nc.sync.dma_start(out=mbdT_d[:, :], in_=mbdT_sb[:])
    nc.compile()
    return nc


_CACHE = {}


def _build_cached():
    if "nc" not in _CACHE:
        _CACHE["nc"] = build()
    return _CACHE["nc"]


def _selector() -> np.ndarray:
    sel = np.zeros((NT, PT, O), np.float32)
    for t in range(NT):
        for p in range(PT):
            sel[t, p, (t * PT + p) % O] = 1.0
    return sel.astype(np.float16)


def make_in_maps(x: np.ndarray, W: np.ndarray):
    xT = np.ascontiguousarray(x.T.astype(np.float16))  # [IN, B]
    # k-major proj.T rows: row p corresponds to (o = p % O, k = p // O),
    # i.e. W row o*K + k
    perm = np.array([(p % O) * K + p // O for p in range(OK)], np.int64)
    wTk = np.ascontiguousarray(W.T.astype(np.float16)[:, perm])  # [IN, OK]
    sel = _selector()
    # partition-contiguous layout: [128, NIN*cols] so each DMA descriptor
    # covers a full per-partition contiguous run
    def prep(a, cols):
        return np.ascontiguousarray(
            a.reshape(NIN, 128, cols).transpose(1, 0, 2).reshape(128, NIN * cols)
        )

    wprep = prep(wTk, OK)
    in_maps = []
    for r in range(NCORES):
        in_maps.append(
            {
                "xT": prep(np.roll(xT, -BL * r, axis=1), B),
                "wT": wprep,
                "sel": sel,
            }
        )
    return in_maps


def run(x, W, trace=False, **kw):
    nc = _build_cached()
    in_maps = make_in_maps(x, W)
    return run_bass_kernel_spmd(
        nc, in_maps, core_ids=list(range(NCORES)), trace=trace, **kw
    )


def kernel(x: np.ndarray, W: np.ndarray) -> np.ndarray:
    x = np.asarray(x, np.float32)
    W = np.asarray(W, np.float32)
    res = run(x, W, trace=False)
    mbd = np.empty((B, O), np.float32)
    for r in range(NCORES):
        mbd[BL * r : BL * (r + 1), :] = res.results[r]["mbdT"].T
    mbd -= 1.0
    return np.concatenate([x, mbd], axis=1)
